# revision 1
# baseline (speedup 1.0000x reference)
"""Trainium2 Bass kernel for nn_DendriteInput (masked linear + per-row top-k mask).

Contract: kernel(**inputs) -> np.ndarray takes FULL inputs
  x[8192,2048] f32, weight[8192,2048] f32, bias[8192] f32,
  duty_cycle[8192] f32, weight_mask[8192,2048] bool
returns FULL output [8192,8192] f32 = y * topk_mask(y*boost, K=819) per row.

Sharding: data-parallel over batch rows; 8 cores x 1024 rows each;
weight/mask/bias/duty replicated. Per core:
  P0a: boost=exp(0.2-2*dc); x -> xT via PE transpose; row-norm warm brackets
  P0b: wT = (w*mask)^T via PE transpose -> DRAM scratch
  P1:  y = x@wT + bias (PSUM-accumulated matmuls, bias via K=1 ones matmul);
       u = 1 - y*boost streamed to DRAM alongside y
  P2:  per-row threshold search on u (warm-started bracketed secant with
       fused-count tensor_scalar/accum on DVE + Sign/accum on ACT),
       exact min-extraction fixup rounds, final mask out = (u<Th)*y
"""
import sys
sys.path.insert(0, '/opt/trn_rl_repo')
import numpy as np

import concourse.bass as bass
import concourse.tile as tile
from concourse import bacc, mybir
from concourse.bass_utils import run_bass_kernel_spmd

AF = mybir.ActivationFunctionType
OP = mybir.AluOpType
dt = mybir.dt
F32 = dt.float32

IN_DIM = 2048
N_DEN = 8192
BATCH = 8192
K_WIN = 819
N_CORES = 8
BOOST_STRENGTH = 2.0
PERCENT_ON = 0.1

C_U = 1.0          # u = C_U - boosted; Sterbenz-exact near threshold ~0.55
C_LO = 0.0112      # warm bracket: thr in [C_LO, C_HI] * ||x_row||
C_HI = 0.0142
DVE_COLS = 5120    # count-pass column split DVE vs ACT


def build_kernel(n_rows=1024, t_secant=12, r_fixup=4, dtype_path="f32",
                 phases="xw12", repeats=1):
    assert n_rows % 128 == 0
    nbt = n_rows // 128
    NB = N_DEN // 512
    ND = IN_DIM // 128
    ACT_COLS = N_DEN - DVE_COLS

    nc = bacc.Bacc("TRN2", target_bir_lowering=False, debug=False,
                   num_devices=N_CORES)

    x_ap = nc.dram_tensor("x", [n_rows, IN_DIM], F32, kind="ExternalInput").ap()
    w_ap = nc.dram_tensor("weight", [N_DEN, IN_DIM], F32, kind="ExternalInput").ap()
    b_ap = nc.dram_tensor("bias", [1, N_DEN], F32, kind="ExternalInput").ap()
    dc_ap = nc.dram_tensor("duty_cycle", [1, N_DEN], F32, kind="ExternalInput").ap()
    m_ap = nc.dram_tensor("weight_mask", [N_DEN, IN_DIM], dt.uint8,
                          kind="ExternalInput").ap()
    id_ap = nc.dram_tensor("ident", [128, 128], F32, kind="ExternalInput").ap()
    nc.dram_tensor("chain", [1, 1], F32, kind="ExternalInput").ap()
    out_ap = nc.dram_tensor("out", [n_rows, N_DEN], F32, kind="ExternalOutput").ap()

    with tile.TileContext(nc) as tc:
        with tc.tile_pool(name="dram", bufs=1, space="DRAM") as dram_pool:
            y_dram = dram_pool.tile([n_rows, N_DEN], F32)
            u_dram = dram_pool.tile([n_rows, N_DEN], F32)
            boost_dram = dram_pool.tile([1, N_DEN], F32)

            for _rep in range(repeats):
                # warm-start state: tiny, spans all phases
                with tc.tile_pool(name="warm", bufs=1) as warm:
                    th0 = warm.tile([128, nbt], F32)
                    tl0 = warm.tile([128, nbt], F32)

                    # ---------- P0 + P1 (matmul pipeline) ----------
                    with tc.tile_pool(name="mmpersist", bufs=1) as mmp:
                        ident = mmp.tile([128, 128], F32)
                        nc.sync.dma_start(ident[:], id_ap[:])
                        ones1 = mmp.tile([1, 128], F32)
                        nc.vector.memset(ones1[:], 1.0)
                        xT = [mmp.tile([128, n_rows], F32, tag=f"xT{j}", name=f"xT{j}")
                              for j in range(ND)]

                        # ----- P0a-pre: boost -----
                        with tc.tile_pool(name="pboost", bufs=2) as pboost:
                            dcol = pboost.tile([1, N_DEN], F32, tag="bchain")
                            nc.sync.dma_start(dcol[:], dc_ap[:])
                            bst = pboost.tile([1, N_DEN], F32, tag="bchain")
                            nc.scalar.activation(bst[:], dcol[:], AF.Exp,
                                                 bias=0.0, scale=-BOOST_STRENGTH)
                            nbst = pboost.tile([1, N_DEN], F32, tag="bchain")
                            nc.vector.tensor_scalar_mul(
                                nbst[:], bst[:],
                                -float(np.exp(BOOST_STRENGTH * PERCENT_ON)))
                            nc.sync.dma_start(boost_dram[:], nbst[:])

                        # ----- P0a: x prep -----
                        with tc.tile_pool(name="p0a", bufs=2) as p0a, \
                             tc.tile_pool(name="p0a_ps", bufs=4, space="PSUM") as p0a_ps:
                            for i in range(nbt):
                                xt = p0a.tile([128, IN_DIM], F32, tag="xt")
                                nc.sync.dma_start(xt[:], x_ap[i * 128:(i + 1) * 128, :])
                                junk = p0a.tile([128, IN_DIM], F32, tag="xjunk")
                                ssq = p0a.tile([128, 1], F32, tag="xssq")
                                nc.vector.scalar_tensor_tensor(
                                    junk[:], xt[:], 1.0, xt[:],
                                    OP.bypass, OP.mult, accum_out=ssq[:])
                                xn = p0a.tile([128, 1], F32, tag="xn")
                                nc.scalar.activation(xn[:], ssq[:], AF.Sqrt)
                                nc.vector.tensor_scalar(th0[:, i:i + 1], xn[:],
                                                        -C_LO, C_U, OP.mult, OP.add)
                                nc.vector.tensor_scalar(tl0[:, i:i + 1], xn[:],
                                                        -C_HI, C_U, OP.mult, OP.add)
                                for j in range(ND):
                                    pst = p0a_ps.tile([128, 128], F32, tag="xps")
                                    nc.tensor.transpose(
                                        pst[:], xt[:, j * 128:(j + 1) * 128], ident[:])
                                    nc.scalar.copy(xT[j][:, i * 128:(i + 1) * 128],
                                                   pst[:])

                        # ----- P1: fused wT-prep + matmul (per n_block) -----
                        # w rows for block nb are transposed into SBUF stage tiles
                        # and consumed directly as matmul rhs (no wT DRAM trip).
                        with tc.tile_pool(name="p1w", bufs=3) as p1w, \
                             tc.tile_pool(name="p1st", bufs=2) as p1st, \
                             tc.tile_pool(name="p1b", bufs=4) as p1b, \
                             tc.tile_pool(name="p1tps", bufs=4, space="PSUM") as p1tps, \
                             tc.tile_pool(name="p1ps", bufs=3, space="PSUM") as p1ps:
                            for nb in range(NB if "1" in phases else 0):
                                stage = p1st.tile([128, ND, 512], F32, tag="stage")
                                nbst = p1w.tile([128, 512], F32, tag="nbst")
                                nc.sync.dma_start(
                                    nbst[:],
                                    boost_dram[0:1, nb * 512:(nb + 1) * 512]
                                    .broadcast_to([128, 512]))
                                for ns in range(4):
                                    nt = nb * 4 + ns
                                    for dh in range(2):
                                        DH = IN_DIM // 2
                                        wt = p1w.tile([128, DH], F32, tag="wt")
                                        nc.sync.dma_start(
                                            wt[:], w_ap[nt * 128:(nt + 1) * 128,
                                                        dh * DH:(dh + 1) * DH])
                                        mt = p1w.tile([128, DH], F32, tag="mt")
                                        nc.gpsimd.dma_start(
                                            mt[:], m_ap[nt * 128:(nt + 1) * 128,
                                                        dh * DH:(dh + 1) * DH])
                                        wm = p1w.tile([128, DH], F32, tag="wm")
                                        nc.vector.tensor_mul(wm[:], wt[:], mt[:])
                                        for dd in range(ND // 2):
                                            d = dh * (ND // 2) + dd
                                            pst = p1tps.tile([128, 128], F32,
                                                             tag="wps")
                                            nc.tensor.transpose(
                                                pst[:],
                                                wm[:, dd * 128:(dd + 1) * 128],
                                                ident[:])
                                            nc.scalar.copy(
                                                stage[:, d,
                                                      ns * 128:(ns + 1) * 128],
                                                pst[:])
                                bias_nb = p1w.tile([1, 512], F32, tag="bias_nb")
                                nc.sync.dma_start(
                                    bias_nb[:], b_ap[0:1, nb * 512:(nb + 1) * 512])
                                for i in range(nbt):
                                    ps = p1ps.tile([128, 512], F32, tag="yps")
                                    nc.tensor.matmul(
                                        ps[:], ones1[:], bias_nb[:],
                                        start=True, stop=False)
                                    for d in range(ND):
                                        nc.tensor.matmul(
                                            ps[:], xT[d][:, i * 128:(i + 1) * 128],
                                            stage[:, d, :], start=False,
                                            stop=(d == ND - 1))
                                    yb = p1b.tile([128, 512], F32, tag="yb")
                                    nc.scalar.copy(yb[:], ps[:])
                                    nc.sync.dma_start(
                                        y_dram[i * 128:(i + 1) * 128,
                                               nb * 512:(nb + 1) * 512], yb[:])
                                    ub = p1b.tile([128, 512], F32, tag="ub")
                                    nc.vector.tensor_mul(ub[:], ps[:], nbst[:])
                                    ub2 = p1b.tile([128, 512], F32, tag="ub2")
                                    nc.vector.tensor_scalar_add(ub2[:], ub[:], C_U)
                                    nc.sync.dma_start(
                                        u_dram[i * 128:(i + 1) * 128,
                                               nb * 512:(nb + 1) * 512], ub2[:])

                    # ---------- P2: threshold search + mask ----------
                    with tc.tile_pool(name="p2", bufs=1) as p2, \
                         tc.tile_pool(name="p2s", bufs=2) as p2s:
                        fh = p2.tile([128, nbt], F32)
                        fl = p2.tile([128, nbt], F32)
                        Th = p2.tile([128, nbt], F32)
                        Tl = p2.tile([128, nbt], F32)
                        nc.vector.tensor_copy(Th[:], th0[:])
                        nc.vector.tensor_copy(Tl[:], tl0[:])

                        # process b_tiles in pairs: big passes per tile, small
                        # vector math batched [128, G] per pair
                        i = 0
                        while i < (nbt if "2" in phases else 0):
                            G = min(2, nbt - i)
                            us = []
                            for j in range(G):
                                uj = p2s.tile([128, N_DEN], F32, tag=f"u{j}",
                                              bufs=1, name=f"u{j}")
                                nc.sync.dma_start(
                                    uj[:],
                                    u_dram[(i + j) * 128:(i + j + 1) * 128, :])
                                us.append(uj)
                            jd = p2s.tile([128, DVE_COLS], dt.bfloat16, tag="jd",
                                          bufs=1)
                            ja = p2s.tile([128, ACT_COLS], dt.bfloat16, tag="ja",
                                          bufs=1)
                            cd = p2s.tile([128, G], F32, tag="cd")
                            sa = p2s.tile([128, G], F32, tag="sa")
                            ThP = Th[:, i:i + G]
                            TlP = Tl[:, i:i + G]
                            fhP = fh[:, i:i + G]
                            flP = fl[:, i:i + G]

                            def count_pair(tgt_cnt, thr_ap):
                                # thr_ap: [128, G]; counts #(u_j < thr_j) -> tgt
                                nthr = p2s.tile([128, G], F32, tag="nthr")
                                nc.scalar.activation(nthr[:], thr_ap, AF.Copy,
                                                     bias=0.0, scale=-1.0)
                                for j in range(G):
                                    nc.vector.tensor_scalar(
                                        jd[:], us[j][:, 0:DVE_COLS],
                                        thr_ap[:, j:j + 1], None,
                                        OP.is_lt, OP.add,
                                        accum_out=cd[:, j:j + 1])
                                    nc.scalar.activation(
                                        ja[:], us[j][:, DVE_COLS:], AF.Sign,
                                        bias=nthr[:, j:j + 1], scale=1.0,
                                        accum_out=sa[:, j:j + 1])
                                t1 = p2s.tile([128, G], F32, tag="t1")
                                nc.scalar.activation(t1[:], sa[:], AF.Copy,
                                                     bias=float(ACT_COLS * 0.5),
                                                     scale=-0.5)
                                nc.vector.tensor_add(tgt_cnt, cd[:], t1[:])

                            count_pair(fhP, ThP)
                            count_pair(flP, TlP)

                            for it in range(t_secant):
                                num = p2s.tile([128, G], F32, tag="num")
                                den = p2s.tile([128, G], F32, tag="den")
                                rcp = p2s.tile([128, G], F32, tag="rcp")
                                tt = p2s.tile([128, G], F32, tag="tt")
                                tc_ = p2s.tile([128, G], F32, tag="tc_")
                                dtl = p2s.tile([128, G], F32, tag="dtl")
                                tdl = p2s.tile([128, G], F32, tag="tdl")
                                mid = p2s.tile([128, G], F32, tag="mid")
                                cnt = p2s.tile([128, G], F32, tag="cnt")
                                nc.vector.tensor_scalar(num[:], flP, -1.0,
                                                        K_WIN - 0.5, OP.mult, OP.add)
                                nc.vector.tensor_sub(den[:], fhP, flP)
                                nc.vector.reciprocal(rcp[:], den[:])
                                nc.vector.tensor_mul(tt[:], num[:], rcp[:])
                                nc.vector.tensor_scalar(tc_[:], tt[:], 0.02, 0.98,
                                                        OP.max, OP.min)
                                nc.vector.tensor_sub(dtl[:], ThP, TlP)
                                nc.vector.tensor_mul(tdl[:], tc_[:], dtl[:])
                                nc.vector.tensor_add(mid[:], TlP, tdl[:])
                                count_pair(cnt[:], mid[:])
                                ind = p2s.tile([128, G], dt.int32, tag="ind")
                                indc = p2s.tile([128, G], dt.int32, tag="indc")
                                nc.vector.tensor_scalar(ind[:], cnt[:],
                                                        float(K_WIN), None, OP.is_ge)
                                nc.vector.tensor_scalar(indc[:], cnt[:],
                                                        float(K_WIN), None, OP.is_lt)
                                nc.vector.copy_predicated(ThP, ind[:], mid[:])
                                nc.vector.copy_predicated(fhP, ind[:], cnt[:])
                                nc.vector.copy_predicated(TlP, indc[:], mid[:])
                                nc.vector.copy_predicated(flP, indc[:], cnt[:])

                            # fixup: one masked pass + blockwise max chain:
                            # up to r_fixup exact drops of the largest
                            # candidates below Th per tile
                            scr = p2s.tile([128, N_DEN], F32, tag="scr", bufs=1)
                            NBLK = 64
                            for j in range(G):
                                ThJ = ThP[:, j:j + 1]
                                fhJ = fhP[:, j:j + 1]
                                nc.vector.scalar_tensor_tensor(
                                    scr[:], us[j][:], ThJ, us[j][:],
                                    OP.is_lt, OP.mult)
                                bmax = p2s.tile([128, NBLK], F32, tag="bmax")
                                nc.vector.reduce_max(
                                    bmax[:],
                                    scr[:].rearrange("p (b c) -> p b c", b=NBLK),
                                    axis=mybir.AxisListType.X)
                                bcur = bmax
                                for r in range(r_fixup):
                                    m = p2s.tile([128, 1], F32, tag=f"m{r}",
                                                 name=f"m{r}")
                                    nc.vector.reduce_max(
                                        m[:], bcur[:],
                                        axis=mybir.AxisListType.X)
                                    need = p2s.tile([128, 1], dt.int32,
                                                    tag="need")
                                    nc.vector.tensor_scalar(
                                        need[:], fhJ, float(K_WIN + r), None,
                                        OP.is_gt)
                                    nc.vector.copy_predicated(ThJ, need[:], m[:])
                                    if r + 1 < r_fixup:
                                        bnew = p2s.tile([128, NBLK], F32,
                                                        tag=f"bm{r}",
                                                        name=f"bm{r}")
                                        nc.vector.scalar_tensor_tensor(
                                            bnew[:], bcur[:], m[:], bcur[:],
                                            OP.is_lt, OP.mult)
                                        bcur = bnew
                                # fh -= clamp(excess, 0, r_fixup)
                                exc = p2s.tile([128, 1], F32, tag="exc")
                                nc.vector.tensor_scalar(
                                    exc[:], fhJ, -float(K_WIN),
                                    float(r_fixup), OP.add, OP.min)
                                ex0 = p2s.tile([128, 1], F32, tag="ex0")
                                nc.vector.tensor_scalar(ex0[:], exc[:], 0.0,
                                                        None, OP.max)
                                nc.vector.tensor_sub(fhJ, fhJ, ex0[:])

                            for j in range(G):
                                yst = p2s.tile([128, N_DEN], F32, tag="yst", bufs=1)
                                nc.sync.dma_start(
                                    yst[:],
                                    y_dram[(i + j) * 128:(i + j + 1) * 128, :])
                                outb = p2s.tile([128, N_DEN], F32, tag="outb",
                                                bufs=1)
                                nc.vector.scalar_tensor_tensor(
                                    outb[:], us[j][:], ThP[:, j:j + 1], yst[:],
                                    OP.is_lt, OP.mult)
                                nc.sync.dma_start(
                                    out_ap[(i + j) * 128:(i + j + 1) * 128, :],
                                    outb[:])
                            i += G

    nc.compile()
    return nc


_BUILT = {}


def _get_built(n_rows=1024, **kw):
    key = (n_rows, tuple(sorted(kw.items())))
    if key not in _BUILT:
        _BUILT[key] = build_kernel(n_rows=n_rows, **kw)
    return _BUILT[key]


def kernel(x, weight, bias, duty_cycle, weight_mask):
    x = np.ascontiguousarray(np.asarray(x, dtype=np.float32))
    weight = np.ascontiguousarray(np.asarray(weight, dtype=np.float32))
    bias = np.ascontiguousarray(np.asarray(bias, dtype=np.float32)).reshape(1, -1)
    duty_cycle = np.ascontiguousarray(
        np.asarray(duty_cycle, dtype=np.float32)).reshape(1, -1)
    mask_u8 = np.ascontiguousarray(np.asarray(weight_mask).astype(np.uint8))
    ident = np.eye(128, dtype=np.float32)

    rows = x.shape[0] // N_CORES
    nc = _get_built(n_rows=rows)
    in_maps = []
    for c in range(N_CORES):
        in_maps.append({
            "x": x[c * rows:(c + 1) * rows],
            "weight": weight,
            "bias": bias,
            "duty_cycle": duty_cycle,
            "weight_mask": mask_u8,
            "ident": ident,
            "chain": np.zeros((1, 1), np.float32),
        })
    res = run_bass_kernel_spmd(nc, in_maps, core_ids=list(range(N_CORES)))
    return np.concatenate([res.results[c]["out"] for c in range(N_CORES)], axis=0)

